# revision 30
# baseline (speedup 1.0000x reference)
"""TRN2 Bass kernel: 2D parallel-beam backprojection (nn_Backprojection).

Input  x: (32, 1, 720, 1024) f32 sinogram  (Z=32 slices, 720 views, 1024 det bins)
Output:   (32, 1, 512, 512) f32 volume.

Sharding over 8 NeuronCores: 2 z-halves x 4 view-quarters.
  core c: z block = 16 slices (c // 4), views = 180 (c % 4), padded to 184.
Host sums the 4 partial volumes per z-half.

Per-core algorithm, iterating over (y-row block of 512 px, view-group of 8):
  - Detector data is packed on host as u32 = (bf16 p, bf16 dp) with
    dp[d] = p[d+1]-p[d], so ONE gathered u32 per (view, pixel) provides the
    full lerp input for 16 z at once (partition layout p = vslot*16 + z).
  - Pool indirect_copy gathers gout[p, x] = q[p, i0(v,x,y)]  (u32 units).
  - PE broadcasts the w row (w8 [8,512] bf16) to all 128 partitions via a
    one-hot matmul -> w_ps [128,512] f32 (PSUM).
  - ACT copies w_ps into the odd bf16 slots of wfull [128,1024] (even slots
    are constant 1.0), so wfull = interleaved (1, w).
  - DVE multiplies gout(bf16 view) * wfull -> tmp = (p, w*dp) at 2x rate.
  - PE reduces the 8 view-slots with a 0/1 matrix into c_ps [16, 1024]
    (pairs still interleaved), accumulating over all 23 view groups in PSUM.
  - ACT+DVE drain c_ps per block (PSUM allows only one PSUM operand per
    instruction): ACT copies even slots, DVE adds odd slots; DMA out every
    8 blocks.
Cross-engine deps use ring buffers (gout ring-8; the rest lag-2). DMA
completion semaphores are parity-split because completions of concurrently
in-flight DMAs are not ordered on this runtime.

The lerp g0*(1-w) + g1*w == p[i0] + w*dp[i0] exactly (up to bf16 rounding,
well within the 2e-2 gate). t is always inside (150, 873): no masking.

Measured on the axon-tunneled trn2 runtime: indirect_copy has a fixed
~14.5us per-instruction cost (and a 512-index ISA cap), so the kernel is
gather-instruction-bound: 11776 gathers/core x 14.56us ~= 171ms. All other
engines (DVE/PE/ACT/DMA) hide completely under that cadence.
"""
import sys

sys.path.insert(0, "/opt/trn_rl_repo")

import numpy as np
import ml_dtypes

NIMG = 512
NDCT = 1024
NVIEW = 720
NZ = 32

NCORES = 8
ZH = 2                 # z halves
VQ = 4                 # view quarters
ZPC = NZ // ZH         # 16 z per core
VPC = NVIEW // VQ      # 180 views per core
VPAD = 184             # padded to a multiple of 8
NG = VPAD // 8         # 23 view groups of 8
NPIX = NIMG * NIMG
NPB = NIMG             # blocks per group = one y row (512 px) each
NITER = NPB * NG       # 11776, iter i = b*NG + g
P = 128
CH = 16                # iterations per idx/w8 DMA chunk
NCHUNK = NITER // CH   # 736
OB = 8                 # blocks per output DMA
SCALE = np.float32(np.pi / NVIEW)


def _build():
    import concourse.bass as bass
    import concourse.mybir as mybir
    from concourse.alu_op_type import AluOpType

    f32 = mybir.dt.float32
    bf16 = mybir.dt.bfloat16
    u32 = mybir.dt.uint32
    u16 = mybir.dt.uint16
    Copy = mybir.ActivationFunctionType.Copy

    nc = bass.Bass()
    q_d = nc.declare_dram_parameter("q", [NG, P, NDCT], u32, isOutput=False)
    idx_d = nc.declare_dram_parameter("idx", [P, NITER, 32], u16, isOutput=False)
    w8_d = nc.declare_dram_parameter("w8", [8, NITER, 512], bf16, isOutput=False)
    oh8_d = nc.declare_dram_parameter("oh8", [8, P], bf16, isOutput=False)
    sel_d = nc.declare_dram_parameter("sel", [P, 16], bf16, isOutput=False)
    out_d = nc.declare_dram_parameter("out", [ZPC, NPIX], f32, isOutput=True)

    from contextlib import ExitStack

    with ExitStack() as ctx:
        q_s = ctx.enter_context(nc.sbuf_tensor("q_s", [P, NG * NDCT], u32))
        idx_s = ctx.enter_context(nc.sbuf_tensor("idx_s", [P, 2 * CH * 32], u16))
        w8_s = ctx.enter_context(nc.sbuf_tensor("w8_s", [8, 2 * CH * 512], bf16))
        oh8 = ctx.enter_context(nc.sbuf_tensor("oh8_s", [8, P], bf16))
        sel = ctx.enter_context(nc.sbuf_tensor("sel_s", [P, 16], bf16))
        GR = 8   # gout ring depth (hides Pool<->DVE semaphore latency)
        gout_s = ctx.enter_context(nc.sbuf_tensor("gout_s", [P, GR * 512], u32))
        wfull_s = ctx.enter_context(nc.sbuf_tensor("wfull_s", [P, 2 * 1024], bf16))
        tmp_s = ctx.enter_context(nc.sbuf_tensor("tmp_s", [P, 2 * 1024], bf16))
        stage_s = ctx.enter_context(
            nc.sbuf_tensor("stage_s", [ZPC, 2 * OB * 512], f32))
        w_ps0 = ctx.enter_context(nc.psum_tensor("w_ps0", [P, 512], f32))
        w_ps1 = ctx.enter_context(nc.psum_tensor("w_ps1", [P, 512], f32))
        c_ps0 = ctx.enter_context(nc.psum_tensor("c_ps0", [16, 1024], f32))
        c_ps1 = ctx.enter_context(nc.psum_tensor("c_ps1", [16, 1024], f32))
        block = ctx.enter_context(nc.Block())
        qsem = ctx.enter_context(nc.semaphore("qsem"))
        isem0 = ctx.enter_context(nc.semaphore("isem0"))
        isem1 = ctx.enter_context(nc.semaphore("isem1"))
        wsm0 = ctx.enter_context(nc.semaphore("wsm0"))
        wsm1 = ctx.enter_context(nc.semaphore("wsm1"))
        osem0 = ctx.enter_context(nc.semaphore("osem0"))
        osem1 = ctx.enter_context(nc.semaphore("osem1"))
        gsem = ctx.enter_context(nc.semaphore("gsem"))
        bcsem = ctx.enter_context(nc.semaphore("bcsem"))
        csem = ctx.enter_context(nc.semaphore("csem"))
        msem = ctx.enter_context(nc.semaphore("msem"))
        ssem = ctx.enter_context(nc.semaphore("ssem"))
        dnsem = ctx.enter_context(nc.semaphore("dnsem"))
        dcsem = ctx.enter_context(nc.semaphore("dcsem"))
        inisem = ctx.enter_context(nc.semaphore("inisem"))

        w_ps = [w_ps0, w_ps1]
        c_ps = [c_ps0, c_ps1]
        isem = [isem0, isem1]
        wsem8 = [wsm0, wsm1]
        osem = [osem0, osem1]

        @block.sync
        def _(sync):
            # constants + full q preload
            sync.dma_start(out=oh8[:], in_=oh8_d[:]).then_inc(qsem, 16)
            sync.dma_start(out=sel[:], in_=sel_d[:]).then_inc(qsem, 16)
            for g in range(NG):
                sync.dma_start(out=q_s[:, g * NDCT:(g + 1) * NDCT],
                               in_=q_d[g]).then_inc(qsem, 16)
            # streamed idx/w8 chunks with out-DMAs interleaved in FIFO order
            next_k = 0
            for c in range(NCHUNK):
                if c >= 2:
                    sync.wait_ge(gsem, CH * (c - 1))
                sync.dma_start(
                    out=idx_s[:, (c % 2) * CH * 32:((c % 2) + 1) * CH * 32],
                    in_=idx_d[:, c * CH:(c + 1) * CH, :],
                ).then_inc(isem[c % 2], 16)
                if c >= 2:
                    sync.wait_ge(bcsem, CH * (c - 1))
                sync.dma_start(
                    out=w8_s[:, (c % 2) * CH * 512:((c % 2) + 1) * CH * 512],
                    in_=w8_d[:, c * CH:(c + 1) * CH, :],
                ).then_inc(wsem8[c % 2], 16)
                # out-chunk k drains at iter ~ (8k+8)*NG; emit once the chunk
                # stream has passed that point
                while next_k < NPB // OB and \
                        (next_k * OB + OB) * NG + 2 * NG <= (c + 1) * CH:
                    k = next_k
                    sync.wait_ge(dnsem, OB * (k + 1))
                    y0 = k * OB
                    sync.dma_start(
                        out=out_d[:, y0 * NIMG:(y0 + OB) * NIMG],
                        in_=stage_s[:, (k % 2) * OB * 512:
                                    ((k % 2) + 1) * OB * 512],
                    ).then_inc(osem[k % 2], 16)
                    next_k += 1
            while next_k < NPB // OB:
                k = next_k
                sync.wait_ge(dnsem, OB * (k + 1))
                y0 = k * OB
                sync.dma_start(
                    out=out_d[:, y0 * NIMG:(y0 + OB) * NIMG],
                    in_=stage_s[:, (k % 2) * OB * 512:((k % 2) + 1) * OB * 512],
                ).then_inc(osem[k % 2], 16)
                next_k += 1

        @block.gpsimd
        def _(g_eng):
            g_eng.wait_ge(qsem, 16 * (NG + 2))
            for i in range(NITER):
                c, j = divmod(i, CH)
                if j == 0:
                    g_eng.wait_ge(isem[c % 2], 16 * (c // 2 + 1))
                if i >= GR:
                    g_eng.wait_ge(msem, i - GR + 1)   # gout[i%GR] free
                g = i % NG
                data3 = q_s[:, g * NDCT:(g + 1) * NDCT].rearrange(
                    "p (e d) -> p e d", d=1)
                out3 = gout_s[:, (i % GR) * 512:((i % GR) + 1) * 512].rearrange(
                    "p (e d) -> p e d", d=1)
                g_eng.indirect_copy(
                    out=out3, data=data3,
                    idxs=idx_s[:, (c % 2) * CH * 32 + j * 32:
                               (c % 2) * CH * 32 + (j + 1) * 32],
                    i_know_ap_gather_is_preferred=True,
                ).then_inc(gsem, 1)

        @block.tensor
        def _(t_eng):
            t_eng.wait_ge(qsem, 16 * (NG + 2))
            for i in range(NITER):
                b, g = divmod(i, NG)
                c, j = divmod(i, CH)
                # broadcast matmul for iter i
                if j == 0:
                    t_eng.wait_ge(wsem8[c % 2], 16 * (c // 2 + 1))
                if i >= 2:
                    t_eng.wait_ge(csem, i - 1)   # w_ps[i%2] free
                t_eng.matmul(
                    out=w_ps[i % 2][:],
                    lhsT=oh8[:],
                    rhs=w8_s[:, (c % 2) * CH * 512 + j * 512:
                             (c % 2) * CH * 512 + (j + 1) * 512],
                    start=True, stop=True,
                ).then_inc(bcsem, 1)
                # selection matmuls for iter i-1
                if i >= 1:
                    i1 = i - 1
                    b1, g1 = divmod(i1, NG)
                    t_eng.wait_ge(msem, i1 + 1)   # tmp[i1%2] ready
                    if g1 == 0 and b1 >= 2:
                        t_eng.wait_ge(dnsem, b1 - 1)  # c_ps[b1%2] drained
                    tb = tmp_s[:, (i1 % 2) * 1024:((i1 % 2) + 1) * 1024]
                    for h in range(2):
                        mm = t_eng.matmul(
                            out=c_ps[b1 % 2][:, h * 512:(h + 1) * 512],
                            lhsT=sel[:],
                            rhs=tb[:, h * 512:(h + 1) * 512],
                            start=(g1 == 0), stop=(g1 == NG - 1),
                        )
                    mm.then_inc(ssem, 1)
            # final iteration's selection matmuls
            i1 = NITER - 1
            b1, g1 = divmod(i1, NG)
            t_eng.wait_ge(msem, i1 + 1)
            tb = tmp_s[:, (i1 % 2) * 1024:((i1 % 2) + 1) * 1024]
            for h in range(2):
                mm = t_eng.matmul(
                    out=c_ps[b1 % 2][:, h * 512:(h + 1) * 512],
                    lhsT=sel[:],
                    rhs=tb[:, h * 512:(h + 1) * 512],
                    start=(g1 == 0), stop=(g1 == NG - 1),
                )
            mm.then_inc(ssem, 1)

        @block.scalar
        def _(s_eng):
            wf3 = [wfull_s[:, 0:1024].rearrange("p (e d) -> p e d", d=2),
                   wfull_s[:, 1024:2048].rearrange("p (e d) -> p e d", d=2)]
            c3 = [c_ps0[:].rearrange("p (e d) -> p e d", d=2),
                  c_ps1[:].rearrange("p (e d) -> p e d", d=2)]
            s_eng.wait_ge(inisem, 1)
            LAG = 4

            def drain1(b):
                # drain step 1: stage = even slots of c_ps[b%2]
                s_eng.wait_ge(ssem, (b * NG + NG - 1) + 1)
                if b % OB == 0 and b >= 2 * OB:
                    k = b // OB - 2
                    s_eng.wait_ge(osem[k % 2], 16 * (k // 2 + 1))
                ko = ((b // OB) % 2) * OB * 512 + (b % OB) * 512
                s_eng.activation(
                    out=stage_s[:, ko:ko + 512], in_=c3[b % 2][:, :, 0],
                    func=Copy,
                ).then_inc(dcsem, 1)

            for i in range(NITER):
                s_eng.wait_ge(bcsem, i + 1)
                if i >= 2:
                    s_eng.wait_ge(msem, i - 1)   # wfull[i%2] free
                s_eng.activation(
                    out=wf3[i % 2][:, :, 1], in_=w_ps[i % 2][:], func=Copy,
                ).then_inc(csem, 1)
                if i >= LAG and (i - LAG) % NG == NG - 1:
                    drain1((i - LAG) // NG)
            for b in range((NITER - 1 - LAG - (NG - 1)) // NG + 1, NPB):
                drain1(b)

        @block.vector
        def _(v_eng):
            from concourse.alu_op_type import AluOpType as Op
            v_eng.memset(wfull_s[:], 1.0).then_inc(inisem, 1)
            c3 = [c_ps0[:].rearrange("p (e d) -> p e d", d=2),
                  c_ps1[:].rearrange("p (e d) -> p e d", d=2)]
            LAGV = 5

            def drain2(b):
                # drain step 2: stage += odd slots of c_ps[b%2]
                v_eng.wait_ge(dcsem, b + 1)
                ko = ((b // OB) % 2) * OB * 512 + (b % OB) * 512
                v_eng.tensor_tensor(
                    out=stage_s[:, ko:ko + 512],
                    in0=stage_s[:, ko:ko + 512], in1=c3[b % 2][:, :, 1],
                    op=Op.add,
                ).then_inc(dnsem, 1)

            for i in range(NITER):
                v_eng.wait_ge(gsem, i + 1)
                v_eng.wait_ge(csem, i + 1)
                if i >= 2:
                    v_eng.wait_ge(ssem, i - 1)   # tmp[i%2] free
                gb = gout_s[:, (i % GR) * 512:((i % GR) + 1) * 512].bitcast(bf16)
                v_eng.tensor_tensor(
                    out=tmp_s[:, (i % 2) * 1024:((i % 2) + 1) * 1024],
                    in0=gb, in1=wfull_s[:, (i % 2) * 1024:((i % 2) + 1) * 1024],
                    op=Op.mult,
                ).then_inc(msem, 1)
                if i >= LAGV and (i - LAGV) % NG == NG - 1:
                    drain2((i - LAGV) // NG)
            for b in range((NITER - 1 - LAGV - (NG - 1)) // NG + 1, NPB):
                drain2(b)
    return nc


# ---------------- host-side tables ----------------

_TABLE_CACHE = {}   # vq -> (idx_arr, w8_arr)
_CONST_CACHE = {}
_NC_CACHE = [None]
LAST_EXEC_NS = None
LAST_TRACE = None


def _quarter_tables(vq):
    """idx [128, NITER, 32] u16 and w8 [8, NITER, 512] bf16 for view quarter."""
    if vq in _TABLE_CACHE:
        return _TABLE_CACHE[vq]
    v0 = vq * VPC
    thetas = np.arange(NVIEW, dtype=np.float64) * (np.pi / NVIEW)
    xs = (np.arange(NIMG, dtype=np.float32) - (NIMG - 1) / 2.0)
    ys = (np.arange(NIMG, dtype=np.float32) - (NIMG - 1) / 2.0)
    ctr = np.float32((NDCT - 1) / 2.0)

    i0q = np.zeros((VPAD, NIMG, NIMG), np.uint16)
    wq = np.zeros((VPAD, NIMG, NIMG), ml_dtypes.bfloat16)
    for vl in range(VPC):
        v = v0 + vl
        c = np.float32(np.cos(thetas[v]))
        s = np.float32(np.sin(thetas[v]))
        t = xs[None, :] * c + ys[:, None] * s + ctr   # (y, x) f32
        i0 = np.floor(t).astype(np.int32)
        wq[vl] = (t - i0.astype(np.float32)).astype(ml_dtypes.bfloat16)
        i0q[vl] = i0.astype(np.uint16)
    # idx: [g, vs, y, j, r] -> [vs, r, y, g, j] -> [(vs r), (y g), j]
    A = i0q.reshape(NG, 8, NIMG, 32, 16)
    idx_arr = np.ascontiguousarray(A.transpose(1, 4, 2, 0, 3)).reshape(
        P, NITER, 32)
    # w8: [g, vs, y, x] -> [vs, y, g, x]
    B = wq.reshape(NG, 8, NIMG, NIMG)
    w8_arr = np.ascontiguousarray(B.transpose(1, 2, 0, 3)).reshape(
        8, NITER, 512)
    _TABLE_CACHE[vq] = (idx_arr, w8_arr)
    return idx_arr, w8_arr


def _consts():
    if "oh8" not in _CONST_CACHE:
        oh8 = np.zeros((8, P), ml_dtypes.bfloat16)
        for v in range(8):
            oh8[v, 16 * v:16 * (v + 1)] = 1.0
        sel = np.zeros((P, 16), ml_dtypes.bfloat16)
        for p in range(P):
            sel[p, p % 16] = 1.0
        _CONST_CACHE["oh8"] = oh8
        _CONST_CACHE["sel"] = sel
    return _CONST_CACHE["oh8"], _CONST_CACHE["sel"]


def _pack_q(x, zh, vq):
    """q [NG, 128, 1024] u32 for core (zh, vq): low = bf16(p), high = bf16(dp)."""
    z0 = zh * ZPC
    v0 = vq * VPC
    p = (x[z0:z0 + ZPC, 0, v0:v0 + VPC, :] * SCALE).astype(ml_dtypes.bfloat16)
    pf = p.astype(np.float32)
    dp = np.zeros_like(pf)
    dp[:, :, :-1] = pf[:, :, 1:] - pf[:, :, :-1]
    dpb = dp.astype(ml_dtypes.bfloat16)
    p_u = p.view(np.uint16).astype(np.uint32)          # (ZPC, VPC, NDCT)
    d_u = dpb.view(np.uint16).astype(np.uint32)
    q = np.zeros((NG, P, NDCT), np.uint32)
    packed = p_u | (d_u << 16)                          # (ZPC, VPC, 1024)
    # partition p = vs*16 + z; group g: view v = g*8 + vs
    for g in range(NG):
        for vs in range(8):
            v = g * 8 + vs
            if v < VPC:
                q[g, 16 * vs:16 * (vs + 1), :] = packed[:, v, :]
    return q


def kernel(x: np.ndarray) -> np.ndarray:
    global LAST_EXEC_NS, LAST_TRACE
    from concourse import bass_utils

    assert x.shape == (NZ, 1, NVIEW, NDCT) and x.dtype == np.float32
    if _NC_CACHE[0] is None:
        _NC_CACHE[0] = _build()
    nc = _NC_CACHE[0]

    oh8, sel = _consts()
    in_maps = []
    for core in range(NCORES):
        zh, vq = divmod(core, VQ)
        idx_arr, w8_arr = _quarter_tables(vq)
        q = _pack_q(x, zh, vq)
        in_maps.append({"q": q, "idx": idx_arr, "w8": w8_arr,
                        "oh8": oh8, "sel": sel})

    br = bass_utils.run_bass_kernel_spmd(nc, in_maps, list(range(NCORES)))
    res = br.results
    LAST_EXEC_NS = br.exec_time_ns
    LAST_TRACE = br.instructions_and_trace

    vols = [res[c]["out"].reshape(ZPC, NIMG, NIMG) for c in range(NCORES)]
    half0 = vols[0] + vols[1] + vols[2] + vols[3]
    half1 = vols[4] + vols[5] + vols[6] + vols[7]
    out = np.concatenate([half0, half1], axis=0)[:, None, :, :]
    return np.ascontiguousarray(out, dtype=np.float32)
